# revision 6
# baseline (speedup 1.0000x reference)
"""Trainium2 Bass kernel for nn_CotLayer (CoT attention layer), v2.

Computation (see reference):
  kemb = relu(grouped_conv3x3(x, Wk, groups=4))
  w1   = relu(We1 @ [x; kemb])            (1x1)
  wbar_k = We2_k @ w1 + be2_k             (per-pixel 3x3 kernel, SP-fold in We2_k)
  xv   = Wv @ x                           (1x1)
  agg  = relu(sum_k shift_k(xv) * wbar_k)
  gap  = mean_{H,W}(agg + kemb)           (AllReduce across 4-core groups)
  attn0 = sigmoid(SE-MLP delta)
  out  = kemb + attn0 * (agg - kemb)

Sharding: 8 cores = (batch b) x (H-quarter q), 64 output rows per core,
1-px halo baked into the input slab host-side; bf16 matmuls, fp32 PSUM.

v2 engine plan (per macro-tile of 4 rows x 256 cols = 1024 px):
  PE : kemb 18 MM-512, w1 4 MM-512, wbar 9 taps as tile_position-paired
       64-contract MMs (concurrent pairs), xv 1 MM-1024 (slab rows, no halo
       recompute).
  ACT: kemb relu (+gap accum), w1 relu, xv PSUM->slab copy,
       7 wbar PSUM->SBUF bias-copies, agg relu (+gap accum).
  DVE: 9 products (2 taps read wbar straight from PSUM via STT, 7 via bf16
       TT@2x), wide in-place add tree (4096+2048+1024+1024), phase-2 STT.
  DMA: x chunks, xv input chunks, odd-column shifted xv copies (SBUF->SBUF),
       bf16->f32 cast stores (SWDGE).
  A dummy AllReduce at startup warms the CC rings so the real one runs ~10us.
"""

import numpy as np
import ml_dtypes
from contextlib import ExitStack

import concourse.bass as bass
import concourse.tile as tile
from concourse import bacc, mybir
from concourse.bass_utils import run_bass_kernel_spmd

F32 = mybir.dt.float32
BF16 = mybir.dt.bfloat16
AL = mybir.AluOpType
AF = mybir.ActivationFunctionType
BF = ml_dtypes.bfloat16

B, C, H, W = 2, 128, 256, 256
KSZ, SP = 3, 8
NCORES = 8
RQ = H // 4          # 64 rows per core
TR = 4               # output rows per macro-tile
NT = RQ // TR        # 16 macro-tiles per core
NPX = TR * W         # 1024 px per macro-tile

STT_TAPS = (0, 1)    # taps whose product reads wbar directly from PSUM


def _prep_weights(inputs):
    Wk = np.asarray(inputs["Wk"], np.float32)
    We1 = np.asarray(inputs["We1"], np.float32)[:, :, 0, 0]
    We2 = np.asarray(inputs["We2"], np.float32)[:, :, 0, 0]
    be2 = np.asarray(inputs["be2"], np.float32)
    Wv = np.asarray(inputs["Wv"], np.float32)[:, :, 0, 0]
    Ws1 = np.asarray(inputs["Ws1"], np.float32)[:, :, 0, 0]
    bs1 = np.asarray(inputs["bs1"], np.float32)
    Ws2 = np.asarray(inputs["Ws2"], np.float32)[:, :, 0, 0]
    bs2 = np.asarray(inputs["bs2"], np.float32)

    # grouped conv as block-diag [C, 9, C]
    wk = np.zeros((C, 9, C), np.float32)
    for t in range(9):
        a, b = divmod(t, 3)
        for g in range(4):
            blk = Wk[32 * g:32 * g + 32, :, a, b]
            wk[32 * g:32 * g + 32, t, 32 * g:32 * g + 32] = blk.T
    # per-tap We2 with the SP-fold replication, taps paired into 64-row groups
    cidx = (np.arange(C) // SP) * 9
    we2 = np.zeros((64, 9, C), np.float32)
    be2k = np.zeros((C, 9), np.float32)
    for t in range(9):
        we2[:, t, :] = We2[cidx + t, :].T
        be2k[:, t] = be2[cidx + t]
    we2p = np.zeros((C, 5, C), np.float32)
    for jj in range(5):
        we2p[0:64, jj, :] = we2[:, 2 * jj, :]
        if jj < 4:
            we2p[64:C, jj, :] = we2[:, 2 * jj + 1, :]
    # w1 weights with duplicated output halves (rows 0-63 == rows 64-127)
    w1x2 = np.concatenate([We1[:, :C].T, We1[:, :C].T], axis=1)
    w1k2 = np.concatenate([We1[:, C:].T, We1[:, C:].T], axis=1)
    # SE weights
    ws1 = np.ascontiguousarray(Ws1.T / float(H * W))            # [C, 64]
    ws2 = np.zeros((64, 2, C), np.float32)
    ws2[:, 0, :] = Ws2[0::2, :].T
    ws2[:, 1, :] = Ws2[1::2, :].T
    bs2r = np.zeros((C, 2), np.float32)
    bs2r[:, 0] = bs2[0::2]
    bs2r[:, 1] = bs2[1::2]
    bs1c = np.zeros((C, 1), np.float32)
    bs1c[0:64, 0] = bs1

    wpack = np.concatenate([
        wk.reshape(C, 9 * C), w1x2, w1k2, we2p.reshape(C, 5 * C), Wv.T,
    ], axis=1).astype(BF)                                       # [C, 17C]
    fpack = np.zeros((C, 332), np.float32)
    fpack[:, 0:9] = be2k
    fpack[:, 9:73] = ws1
    fpack[:, 73:74] = bs1c
    fpack[:, 74:76] = bs2r
    fpack[0:64, 76:332] = ws2.reshape(64, 2 * C)
    return dict(wpack=np.ascontiguousarray(wpack),
                fpack=np.ascontiguousarray(fpack))


def _build_kernel(nc):
    xs = nc.dram_tensor("xs", [C, RQ + 2, W + 2], BF16, kind="ExternalInput")
    wpack_d = nc.dram_tensor("wpack", [C, 17 * C], BF16, kind="ExternalInput")
    fpack_d = nc.dram_tensor("fpack", [C, 332], F32, kind="ExternalInput")
    out_d = nc.dram_tensor("out", [C, RQ * W], F32, kind="ExternalOutput")

    cc_in = nc.dram_tensor("cc_in", [C, 1], F32, kind="Internal")
    cc_out = nc.dram_tensor("cc_out", [C, 1], F32, kind="Internal")
    cc1_in = nc.dram_tensor("cc1_in", [C, 1], F32, kind="Internal")
    cc1_out = nc.dram_tensor("cc1_out", [C, 1], F32, kind="Internal")
    ccw_in = nc.dram_tensor("ccw_in", [C, 1], F32, kind="Internal")
    ccw_out = nc.dram_tensor("ccw_out", [C, 1], F32, kind="Internal")

    with tile.TileContext(nc) as tc, ExitStack() as ctx:
        singles = ctx.enter_context(tc.tile_pool(name="singles", bufs=1))
        xcp = ctx.enter_context(tc.tile_pool(name="xcp", bufs=4))
        xinp = ctx.enter_context(tc.tile_pool(name="xinp", bufs=3))
        oddp = ctx.enter_context(tc.tile_pool(name="oddp", bufs=3))
        w1p = ctx.enter_context(tc.tile_pool(name="w1p", bufs=2))
        wbp = ctx.enter_context(tc.tile_pool(name="wbp", bufs=3))
        prodp = ctx.enter_context(tc.tile_pool(name="prodp", bufs=2))
        outp = ctx.enter_context(tc.tile_pool(name="outp", bufs=4))
        smallp = ctx.enter_context(tc.tile_pool(name="smallp", bufs=1))
        # PSUM: pkp holds kemb/w1 (2 allocs per tile, so kemb(t+1) can start
        # right after kemb(t) drains), pwb rotates the 9 wbar taps + xv
        pkp = ctx.enter_context(tc.tile_pool(name="pkp", bufs=2, space="PSUM"))
        pwb = ctx.enter_context(tc.tile_pool(name="pwb", bufs=2, space="PSUM"))

        wpack_sb = singles.tile([C, 17 * C], BF16, tag="wpack")
        nc.sync.dma_start(wpack_sb, wpack_d.ap())
        fpack_sb = singles.tile([C, 332], F32, tag="fpack")
        nc.sync.dma_start(fpack_sb, fpack_d.ap())

        wk_v = wpack_sb[:, 0:9 * C].rearrange("p (t c) -> p t c", c=C)
        w1x_v = wpack_sb[:, 9 * C:10 * C]
        w1k_v = wpack_sb[:, 10 * C:11 * C]
        we2_v = wpack_sb[:, 11 * C:16 * C].rearrange("p (j c) -> p j c", c=C)
        wv_v = wpack_sb[:, 16 * C:17 * C]
        be2_v = fpack_sb[:, 0:9]
        ws1_v = fpack_sb[:, 9:73]
        bs1_v = fpack_sb[0:64, 73:74]
        bs2_v = fpack_sb[:, 74:76]
        ws2_v = fpack_sb[0:64, 76:332].rearrange("p (j c) -> p j c", c=C)

        kemb_slab = singles.tile([C, NT * NPX], BF16, tag="kemb")
        agg_slab = singles.tile([C, NT * NPX], BF16, tag="agg")
        xv_slab = singles.tile([C, RQ + 2, W + 2], BF16, tag="xvs")
        xin0 = singles.tile([C, 10, W + 2], BF16, tag="xin0")
        slots_k = singles.tile([C, 2 * NT], F32, tag="slotk")
        slots_a = singles.tile([C, NT], F32, tag="slota")
        attn0 = singles.tile([C, 1], F32, tag="attn0")

        # zero the xv column borders (aggregation zero-pad)
        nc.gpsimd.memset(xv_slab[:, :, 0:1], 0.0)
        nc.gpsimd.memset(xv_slab[:, :, W + 1:W + 2], 0.0)

        # pre-warm sigmoid table + dummy AllReduce to warm the CC rings
        warm = smallp.tile([C, 1], F32, tag="warm")
        nc.vector.memset(warm, 0.0)
        nc.scalar.activation(warm, warm, AF.Sigmoid)
        nc.sync.dma_start(ccw_in.ap(), warm)
        nc.gpsimd.collective_compute(
            "AllReduce", AL.add,
            replica_groups=[[0, 1, 2, 3], [4, 5, 6, 7]],
            ins=[ccw_in.ap().opt()],
            outs=[ccw_out.ap().opt()],
        )

        # ---- xv bootstrap: slab rows 0..9 ----
        nc.sync.dma_start(xin0, xs.ap()[:, 0:10, :])
        for (r0, nr) in ((0, 4), (4, 4), (8, 2)):
            pv = pwb.tile([C, NPX], F32, tag="wbmm")
            nc.tensor.matmul(pv[:, 0:nr * W // 2], lhsT=wv_v,
                             rhs=xin0[:, r0:r0 + nr // 2, 1:1 + W],
                             start=True, stop=True)
            nc.tensor.matmul(pv[:, nr * W // 2:nr * W], lhsT=wv_v,
                             rhs=xin0[:, r0 + nr // 2:r0 + nr, 1:1 + W],
                             start=True, stop=True)
            nc.scalar.activation(
                xv_slab[:, r0:r0 + nr, 1:1 + W],
                pv[:, 0:nr * W].rearrange("p (r w) -> p r w", w=W), AF.Copy)

        odd_cur = oddp.tile([C, 6, W], BF16, tag="odd")
        nc.sync.dma_start(odd_cur, xv_slab[:, 0:6, 1:1 + W])

        # ---------------- software-pipelined main loop ----------------
        # kemb/w1 for tile t+1 are computed during tile t's tap/product
        # phase, so the ACT queue never stalls on the kemb->w1->wbar chain.
        def kemb_w1(t, xc):
            pk = pkp.tile([C, NPX], F32, tag="pk")
            kv = kemb_slab[:, t * NPX:(t + 1) * NPX]
            for g2 in range(2):
                for tap in range(9):
                    a, b = divmod(tap, 3)
                    nc.tensor.matmul(
                        pk[:, g2 * 512:(g2 + 1) * 512],
                        lhsT=wk_v[:, tap, :],
                        rhs=xc[:, 2 * g2 + a:2 * g2 + a + 2, b:b + W],
                        start=(tap == 0), stop=(tap == 8),
                    )
                nc.scalar.activation(kv[:, g2 * 512:(g2 + 1) * 512],
                                     pk[:, g2 * 512:(g2 + 1) * 512], AF.Relu,
                                     accum_out=slots_k[:, 2 * t + g2:
                                                       2 * t + g2 + 1])
            pw = pkp.tile([C, NPX], F32, tag="pk")
            w1b = w1p.tile([C, NPX], BF16, tag="w1")
            for g2 in range(2):
                cs = slice(g2 * 512, (g2 + 1) * 512)
                nc.tensor.matmul(pw[:, cs], lhsT=w1x_v,
                                 rhs=xc[:, 1 + 2 * g2:3 + 2 * g2, 1:1 + W],
                                 start=True, stop=False)
                nc.tensor.matmul(pw[:, cs], lhsT=w1k_v, rhs=kv[:, cs],
                                 start=False, stop=True)
                nc.scalar.activation(w1b[:, cs], pw[:, cs], AF.Relu)
            return w1b

        xc = xcp.tile([C, TR + 2, W + 2], BF16, tag="xc")
        nc.sync.dma_start(xc, xs.ap()[:, 0:TR + 2, :])
        w1b_cur = kemb_w1(0, xc)

        for t in range(NT):
            if t < NT - 1:
                xc = xcp.tile([C, TR + 2, W + 2], BF16, tag="xc")
                nc.sync.dma_start(
                    xc, xs.ap()[:, TR * (t + 1):TR * (t + 1) + TR + 2, :])
                w1b_next = kemb_w1(t + 1, xc)

            # xv production for slab rows 4t+10..4t+13
            if t < 14:
                xin = xinp.tile([C, 4, W + 2], BF16, tag="xin")
                nc.sync.dma_start(xin, xs.ap()[:, 4 * t + 10:4 * t + 14, :])
                pv = pwb.tile([C, NPX], F32, tag="wbmm")
                nc.tensor.matmul(pv[:, 0:512], lhsT=wv_v,
                                 rhs=xin[:, 0:2, 1:1 + W], start=True, stop=True)
                nc.tensor.matmul(pv[:, 512:1024], lhsT=wv_v,
                                 rhs=xin[:, 2:4, 1:1 + W], start=True, stop=True)
                nc.scalar.activation(
                    xv_slab[:, 4 * t + 10:4 * t + 14, 1:1 + W],
                    pv.rearrange("p (r w) -> p r w", w=W), AF.Copy)

            # wbar taps (tile_position-paired 64-contract MMs) + products
            prod = prodp.tile([C, 9216], BF16, tag="prod")

            def xsrc(tap):
                a, b = divmod(tap, 3)
                if b == 1:
                    return odd_cur[:, a:a + TR, :]
                return xv_slab[:, 4 * t + a:4 * t + a + TR, b:b + W]

            for jj in range(5):
                taps = [2 * jj] + ([2 * jj + 1] if jj < 4 else [])
                pbs = {}
                for ti, tap in enumerate(taps):
                    pb = pwb.tile([C, NPX], F32, tag="wbmm")
                    lo = 64 * ti
                    for h in range(2):
                        cs = slice(512 * h, 512 * h + 512)
                        nc.tensor.matmul(
                            pb[:, cs],
                            lhsT=we2_v[lo:lo + 64, jj, :],
                            rhs=w1b_cur[lo:lo + 64, cs],
                            start=True, stop=True,
                            tile_position=(lo, 0))
                    pbs[tap] = pb
                for tap in taps:
                    pview = prod[:, tap * NPX:(tap + 1) * NPX] \
                        .rearrange("p (r w) -> p r w", w=W)
                    pb3 = pbs[tap].rearrange("p (r w) -> p r w", w=W)
                    if tap in STT_TAPS:
                        nc.vector.scalar_tensor_tensor(
                            pview, pb3, be2_v[:, tap:tap + 1], xsrc(tap),
                            AL.add, AL.mult)
                    else:
                        wb = wbp.tile([C, NPX], BF16, tag="wb")
                        nc.scalar.activation(wb, pbs[tap], AF.Identity,
                                             bias=be2_v[:, tap:tap + 1])
                        nc.vector.tensor_tensor(
                            pview, wb.rearrange("p (r w) -> p r w", w=W),
                            xsrc(tap), AL.mult)

            # wide in-place add tree + relu(+accum)
            nc.vector.tensor_tensor(prod[:, 0:4096], prod[:, 0:4096],
                                    prod[:, 4096:8192], AL.add)
            nc.vector.tensor_tensor(prod[:, 0:2048], prod[:, 0:2048],
                                    prod[:, 2048:4096], AL.add)
            nc.vector.tensor_tensor(prod[:, 0:1024], prod[:, 0:1024],
                                    prod[:, 1024:2048], AL.add)
            nc.vector.tensor_tensor(prod[:, 1024:2048], prod[:, 0:1024],
                                    prod[:, 8192:9216], AL.add)
            av = agg_slab[:, t * NPX:(t + 1) * NPX]
            nc.scalar.activation(av, prod[:, 1024:2048], AF.Relu,
                                 accum_out=slots_a[:, t:t + 1])

            # d = agg - kemb, chunked in-loop so the final gap chain is short
            if t in (3, 7, 11):
                c0 = (t - 3) * 1024
                nc.vector.tensor_tensor(
                    agg_slab[:, c0:c0 + 4096], agg_slab[:, c0:c0 + 4096],
                    kemb_slab[:, c0:c0 + 4096], AL.subtract)
            elif t == 14:
                nc.vector.tensor_tensor(
                    agg_slab[:, 12288:15360], agg_slab[:, 12288:15360],
                    kemb_slab[:, 12288:15360], AL.subtract)
            elif t == 15:
                nc.vector.tensor_tensor(
                    agg_slab[:, 15360:16384], agg_slab[:, 15360:16384],
                    kemb_slab[:, 15360:16384], AL.subtract)

            if t < NT - 1:
                w1b_cur = w1b_next

            if t == 11:
                # early partial-gap AllReduce (tiles 0..11) -- hides peer skew
                sk1 = smallp.tile([C, 1], F32, tag="sk1")
                sa1 = smallp.tile([C, 1], F32, tag="sa1")
                nc.vector.tensor_reduce(sk1, slots_k[:, 0:24],
                                        mybir.AxisListType.X, AL.add)
                nc.vector.tensor_reduce(sa1, slots_a[:, 0:12],
                                        mybir.AxisListType.X, AL.add)
                gap1 = smallp.tile([C, 1], F32, tag="gap1")
                nc.vector.tensor_tensor(gap1, sk1, sa1, AL.add)
                nc.sync.dma_start(cc1_in.ap(), gap1)
                nc.gpsimd.collective_compute(
                    "AllReduce", AL.add,
                    replica_groups=[[0, 1, 2, 3], [4, 5, 6, 7]],
                    ins=[cc1_in.ap().opt()],
                    outs=[cc1_out.ap().opt()],
                )

            # odd-shifted xv copy for next tile
            if t < NT - 1:
                odd_cur = oddp.tile([C, 6, W], BF16, tag="odd")
                nc.sync.dma_start(
                    odd_cur, xv_slab[:, 4 * (t + 1):4 * (t + 1) + 6, 1:1 + W])

        # ---------------- gap remainder + final collective ----------------
        sum_k = smallp.tile([C, 1], F32, tag="sk")
        sum_a = smallp.tile([C, 1], F32, tag="sa")
        nc.vector.tensor_reduce(sum_k, slots_k[:, 24:32], mybir.AxisListType.X,
                                AL.add)
        nc.vector.tensor_reduce(sum_a, slots_a[:, 12:16], mybir.AxisListType.X,
                                AL.add)
        gap = smallp.tile([C, 1], F32, tag="gap")
        nc.vector.tensor_tensor(gap, sum_k, sum_a, AL.add)
        nc.sync.dma_start(cc_in.ap(), gap)
        nc.gpsimd.collective_compute(
            "AllReduce", AL.add,
            replica_groups=[[0, 1, 2, 3], [4, 5, 6, 7]],
            ins=[cc_in.ap().opt()],
            outs=[cc_out.ap().opt()],
        )
        gap2 = smallp.tile([C, 1], F32, tag="gap2")
        nc.sync.dma_start(gap2, cc_out.ap())
        gap1o = smallp.tile([C, 1], F32, tag="gap1o")
        nc.sync.dma_start(gap1o, cc1_out.ap())
        nc.vector.tensor_tensor(gap2, gap2, gap1o, AL.add)

        # ---------------- SE attention (tiny) ----------------
        ph = pkp.tile([64, 1], F32, tag="pk")
        nc.tensor.matmul(ph, lhsT=ws1_v, rhs=gap2, start=True, stop=True)
        hso = smallp.tile([64, 1], F32, tag="h")
        nc.scalar.activation(hso, ph, AF.Relu, bias=bs1_v)
        pa = pkp.tile([C, 2], F32, tag="pk")
        nc.tensor.matmul(pa[:, 0:1], lhsT=ws2_v[:, 0, :], rhs=hso,
                         start=True, stop=True)
        nc.tensor.matmul(pa[:, 1:2], lhsT=ws2_v[:, 1, :], rhs=hso,
                         start=True, stop=True)
        a01 = smallp.tile([C, 2], F32, tag="a01")
        nc.scalar.activation(a01[:, 0:1], pa[:, 0:1], AF.Identity,
                             bias=bs2_v[:, 0:1])
        nc.scalar.activation(a01[:, 1:2], pa[:, 1:2], AF.Identity,
                             bias=bs2_v[:, 1:2])
        dse = smallp.tile([C, 1], F32, tag="dse")
        nc.vector.tensor_tensor(dse, a01[:, 0:1], a01[:, 1:2], AL.subtract)
        nc.scalar.activation(attn0, dse, AF.Sigmoid)

        # ---------------- phase 2: out = kemb + attn0 * d ----------------
        # agg_slab holds d; scale by attn0 (4x ts, in place), add kemb,
        # store via SWDGE cast bf16->f32
        for c8 in range(8):
            cs = slice(c8 * 2048, (c8 + 1) * 2048)
            nc.vector.tensor_scalar(agg_slab[:, cs], agg_slab[:, cs],
                                    attn0[:, 0:1], None, AL.mult)
            ob = outp.tile([C, 2048], BF16, tag="ob")
            nc.vector.tensor_tensor(ob, agg_slab[:, cs], kemb_slab[:, cs],
                                    AL.add)
            nc.gpsimd.dma_start(out_d.ap()[:, cs], ob)

    return nc


_CACHE = {}


def _get_nc():
    if "nc" not in _CACHE:
        nc = bacc.Bacc("TRN2", target_bir_lowering=False, debug=False,
                       num_devices=NCORES)
        _build_kernel(nc)
        nc.compile()
        _CACHE["nc"] = nc
    return _CACHE["nc"]


def make_in_maps(inputs):
    x = np.asarray(inputs["x"], np.float32)
    wts = _prep_weights(inputs)
    xp = np.pad(x, ((0, 0), (0, 0), (1, 1), (1, 1))).astype(BF)
    in_maps = []
    for core in range(NCORES):
        bb, q = divmod(core, 4)
        slab = np.ascontiguousarray(xp[bb, :, RQ * q:RQ * q + RQ + 2, :])
        m = {"xs": slab}
        m.update(wts)
        in_maps.append(m)
    return in_maps


def kernel(**inputs):
    in_maps = make_in_maps(inputs)
    nc = _get_nc()
    res = run_bass_kernel_spmd(nc, in_maps, core_ids=list(range(NCORES)))
    out = np.empty((B, C, H, W), np.float32)
    for core in range(NCORES):
        bb, q = divmod(core, 4)
        out[bb, :, RQ * q:RQ * q + RQ, :] = \
            res.results[core]["out"].reshape(C, RQ, W)
    return out


# revision 8
# speedup vs baseline: 1.0067x; 1.0067x over previous
"""Trainium2 Bass kernel for nn_CotLayer (CoT attention layer), v6.

Computation (see reference):
  kemb = relu(grouped_conv3x3(x, Wk, groups=4))
  w1   = relu(We1 @ [x; kemb])            (1x1)
  wbar_k = We2_k @ w1 + be2_k             (per-pixel 3x3 kernel, SP-fold in We2_k)
  xv   = Wv @ x                           (1x1)
  agg  = relu(sum_k shift_k(xv) * wbar_k)
  gap  = mean_{H,W}(agg + kemb)           (AllReduce across 4-core groups)
  attn0 = sigmoid(SE-MLP delta)
  out  = kemb + attn0 * (agg - kemb)

Sharding: 8 cores = (batch b) x (H-quarter q), 64 output rows per core,
1-px halo baked into the input slab host-side; bf16 matmuls, fp32 PSUM.

Per macro-tile (4 rows x 256 cols = 1024 px), software-pipelined: tile t's
tap/product/tree phase runs while tile t+1's kemb/w1 matmuls+relus fill in
behind it (taps first in program order so the PE queue serves them early).
  PE : 18 kemb MM-512 (lhsT reused between row-halves), 4 w1 MM-512,
       9 wbar 64-contract MMs in tile_position-paired slots, 2 xv MM-512.
  ACT: kemb/w1 half-relus (+gap accum), xv PSUM->slab copy, agg relu
       (+gap accum), 8 wbar PSUM->SBUF bias copies.
  DVE: 9 products (1 via STT straight from PSUM, 8 via bf16 TT@2x),
       4 interleaved pair-adds + 3-op mini-tree, d=agg-kemb chunks
       (hidden in loop slack), phase-2 scale+add.
  DMA: x chunks, xv input chunks, odd-column shifted xv copies (SBUF->SBUF),
       SWDGE bf16->f32 cast stores.
Collectives: dummy AllReduce at startup warms the CC rings; the gap
AllReduce is split (tiles 0-11 issued early to hide peer skew + remainder).
"""

import numpy as np
import ml_dtypes
from contextlib import ExitStack

import concourse.bass as bass
import concourse.tile as tile
from concourse import bacc, mybir
from concourse.bass_utils import run_bass_kernel_spmd

F32 = mybir.dt.float32
BF16 = mybir.dt.bfloat16
AL = mybir.AluOpType
AF = mybir.ActivationFunctionType
BF = ml_dtypes.bfloat16

B, C, H, W = 2, 128, 256, 256
KSZ, SP = 3, 8
NCORES = 8
RQ = H // 4          # 64 rows per core
TR = 4               # output rows per macro-tile
NT = RQ // TR        # 16 macro-tiles per core
NPX = TR * W         # 1024 px per macro-tile

STT_TAPS = (1,)      # taps whose product reads wbar directly from PSUM
# product slot offsets in the [C, 9216] buffer: pair jj -> (A_jj, B_jj),
# tap8 -> 8192; pair-adds A_jj += B_jj land s_jj contiguously in [0:4096]
SLOT_OFF = {0: 0, 1: 4096, 2: 1024, 3: 5120, 4: 2048, 5: 6144,
            6: 3072, 7: 7168, 8: 8192}


def _prep_weights(inputs):
    Wk = np.asarray(inputs["Wk"], np.float32)
    We1 = np.asarray(inputs["We1"], np.float32)[:, :, 0, 0]
    We2 = np.asarray(inputs["We2"], np.float32)[:, :, 0, 0]
    be2 = np.asarray(inputs["be2"], np.float32)
    Wv = np.asarray(inputs["Wv"], np.float32)[:, :, 0, 0]
    Ws1 = np.asarray(inputs["Ws1"], np.float32)[:, :, 0, 0]
    bs1 = np.asarray(inputs["bs1"], np.float32)
    Ws2 = np.asarray(inputs["Ws2"], np.float32)[:, :, 0, 0]
    bs2 = np.asarray(inputs["bs2"], np.float32)

    # grouped conv as block-diag [C, 9, C]
    wk = np.zeros((C, 9, C), np.float32)
    for t in range(9):
        a, b = divmod(t, 3)
        for g in range(4):
            blk = Wk[32 * g:32 * g + 32, :, a, b]
            wk[32 * g:32 * g + 32, t, 32 * g:32 * g + 32] = blk.T
    # per-tap We2 with the SP-fold replication, taps paired into 64-row groups
    cidx = (np.arange(C) // SP) * 9
    we2 = np.zeros((64, 9, C), np.float32)
    be2k = np.zeros((C, 9), np.float32)
    for t in range(9):
        we2[:, t, :] = We2[cidx + t, :].T
        be2k[:, t] = be2[cidx + t]
    we2p = np.zeros((C, 5, C), np.float32)
    for jj in range(5):
        we2p[0:64, jj, :] = we2[:, 2 * jj, :]
        if jj < 4:
            we2p[64:C, jj, :] = we2[:, 2 * jj + 1, :]
    # w1 weights with duplicated output halves (rows 0-63 == rows 64-127)
    w1x2 = np.concatenate([We1[:, :C].T, We1[:, :C].T], axis=1)
    w1k2 = np.concatenate([We1[:, C:].T, We1[:, C:].T], axis=1)
    # SE weights
    ws1 = np.ascontiguousarray(Ws1.T / float(H * W))            # [C, 64]
    ws2 = np.zeros((64, 2, C), np.float32)
    ws2[:, 0, :] = Ws2[0::2, :].T
    ws2[:, 1, :] = Ws2[1::2, :].T
    bs2r = np.zeros((C, 2), np.float32)
    bs2r[:, 0] = bs2[0::2]
    bs2r[:, 1] = bs2[1::2]
    bs1c = np.zeros((C, 1), np.float32)
    bs1c[0:64, 0] = bs1

    wpack = np.concatenate([
        wk.reshape(C, 9 * C), w1x2, w1k2, we2p.reshape(C, 5 * C), Wv.T,
    ], axis=1).astype(BF)                                       # [C, 17C]
    fpack = np.zeros((C, 332), np.float32)
    fpack[:, 0:9] = be2k
    fpack[:, 9:73] = ws1
    fpack[:, 73:74] = bs1c
    fpack[:, 74:76] = bs2r
    fpack[0:64, 76:332] = ws2.reshape(64, 2 * C)
    return dict(wpack=np.ascontiguousarray(wpack),
                fpack=np.ascontiguousarray(fpack))


def _build_kernel(nc):
    xs = nc.dram_tensor("xs", [C, RQ + 2, W + 2], BF16, kind="ExternalInput")
    wpack_d = nc.dram_tensor("wpack", [C, 17 * C], BF16, kind="ExternalInput")
    fpack_d = nc.dram_tensor("fpack", [C, 332], F32, kind="ExternalInput")
    out_d = nc.dram_tensor("out", [C, RQ * W], F32, kind="ExternalOutput")

    cc_in = nc.dram_tensor("cc_in", [C, 1], F32, kind="Internal")
    cc_out = nc.dram_tensor("cc_out", [C, 1], F32, kind="Internal")
    cc1_in = nc.dram_tensor("cc1_in", [C, 1], F32, kind="Internal")
    cc1_out = nc.dram_tensor("cc1_out", [C, 1], F32, kind="Internal")
    ccw_in = nc.dram_tensor("ccw_in", [C, 1], F32, kind="Internal")
    ccw_out = nc.dram_tensor("ccw_out", [C, 1], F32, kind="Internal")

    with tile.TileContext(nc) as tc, ExitStack() as ctx:
        singles = ctx.enter_context(tc.tile_pool(name="singles", bufs=1))
        xcp = ctx.enter_context(tc.tile_pool(name="xcp", bufs=4))
        xinp = ctx.enter_context(tc.tile_pool(name="xinp", bufs=3))
        oddp = ctx.enter_context(tc.tile_pool(name="oddp", bufs=3))
        w1p = ctx.enter_context(tc.tile_pool(name="w1p", bufs=2))
        wbp = ctx.enter_context(tc.tile_pool(name="wbp", bufs=3))
        prodp = ctx.enter_context(tc.tile_pool(name="prodp", bufs=2))
        outp = ctx.enter_context(tc.tile_pool(name="outp", bufs=4))
        smallp = ctx.enter_context(tc.tile_pool(name="smallp", bufs=1))
        # PSUM: pkp holds kemb/w1 (2 allocs per tile), pwb 9 wbar taps + xv
        pkp = ctx.enter_context(tc.tile_pool(name="pkp", bufs=2, space="PSUM"))
        pwb = ctx.enter_context(tc.tile_pool(name="pwb", bufs=2, space="PSUM"))

        wpack_sb = singles.tile([C, 17 * C], BF16, tag="wpack")
        nc.sync.dma_start(wpack_sb, wpack_d.ap())
        fpack_sb = singles.tile([C, 332], F32, tag="fpack")
        nc.sync.dma_start(fpack_sb, fpack_d.ap())

        wk_v = wpack_sb[:, 0:9 * C].rearrange("p (t c) -> p t c", c=C)
        w1x_v = wpack_sb[:, 9 * C:10 * C]
        w1k_v = wpack_sb[:, 10 * C:11 * C]
        we2_v = wpack_sb[:, 11 * C:16 * C].rearrange("p (j c) -> p j c", c=C)
        wv_v = wpack_sb[:, 16 * C:17 * C]
        be2_v = fpack_sb[:, 0:9]
        ws1_v = fpack_sb[:, 9:73]
        bs1_v = fpack_sb[0:64, 73:74]
        bs2_v = fpack_sb[:, 74:76]
        ws2_v = fpack_sb[0:64, 76:332].rearrange("p (j c) -> p j c", c=C)

        kemb_slab = singles.tile([C, NT * NPX], BF16, tag="kemb")
        agg_slab = singles.tile([C, NT * NPX], BF16, tag="agg")
        xv_slab = singles.tile([C, RQ + 2, W + 2], BF16, tag="xvs")
        xin0 = singles.tile([C, 10, W + 2], BF16, tag="xin0")
        slots_k = singles.tile([C, 2 * NT], F32, tag="slotk")
        slots_a = singles.tile([C, NT], F32, tag="slota")
        dslots = singles.tile([C, 5], F32, tag="dslots")
        attn0 = singles.tile([C, 1], F32, tag="attn0")

        # zero the xv column borders (aggregation zero-pad)
        nc.gpsimd.memset(xv_slab[:, :, 0:1], 0.0)
        nc.gpsimd.memset(xv_slab[:, :, W + 1:W + 2], 0.0)

        # pre-warm sigmoid table + dummy AllReduce to warm the CC rings
        warm = smallp.tile([C, 1], F32, tag="warm")
        nc.vector.memset(warm, 0.0)
        nc.scalar.activation(warm, warm, AF.Sigmoid)
        nc.sync.dma_start(ccw_in.ap(), warm)
        nc.gpsimd.collective_compute(
            "AllReduce", AL.add,
            replica_groups=[[0, 1, 2, 3], [4, 5, 6, 7]],
            ins=[ccw_in.ap().opt()],
            outs=[ccw_out.ap().opt()],
        )

        # ---- xv bootstrap: slab rows 0..9 ----
        nc.sync.dma_start(xin0, xs.ap()[:, 0:10, :])
        for (r0, nr) in ((0, 4), (4, 4), (8, 2)):
            pv = pwb.tile([C, NPX], F32, tag="wbmm")
            nc.tensor.matmul(pv[:, 0:nr * W // 2], lhsT=wv_v,
                             rhs=xin0[:, r0:r0 + nr // 2, 1:1 + W],
                             start=True, stop=True)
            nc.tensor.matmul(pv[:, nr * W // 2:nr * W], lhsT=wv_v,
                             rhs=xin0[:, r0 + nr // 2:r0 + nr, 1:1 + W],
                             start=True, stop=True)
            nc.scalar.activation(
                xv_slab[:, r0:r0 + nr, 1:1 + W],
                pv[:, 0:nr * W].rearrange("p (r w) -> p r w", w=W), AF.Copy)

        odd_cur = oddp.tile([C, 6, W], BF16, tag="odd")
        nc.sync.dma_start(odd_cur, xv_slab[:, 0:6, 1:1 + W])

        # kemb + w1 for one tile (pipelined one tile ahead of the tap phase)
        def kemb_w1(t, xc):
            pk = pkp.tile([C, NPX], F32, tag="pk")
            kv = kemb_slab[:, t * NPX:(t + 1) * NPX]
            for g2 in range(2):
                for tap in range(9):
                    a, b = divmod(tap, 3)
                    nc.tensor.matmul(
                        pk[:, g2 * 512:(g2 + 1) * 512],
                        lhsT=wk_v[:, tap, :],
                        rhs=xc[:, 2 * g2 + a:2 * g2 + a + 2, b:b + W],
                        start=(tap == 0), stop=(tap == 8),
                    )
                nc.scalar.activation(kv[:, g2 * 512:(g2 + 1) * 512],
                                     pk[:, g2 * 512:(g2 + 1) * 512], AF.Relu,
                                     accum_out=slots_k[:, 2 * t + g2:
                                                       2 * t + g2 + 1])
            pw = pkp.tile([C, NPX], F32, tag="pk")
            w1b = w1p.tile([C, NPX], BF16, tag="w1")
            for g2 in range(2):
                cs = slice(g2 * 512, (g2 + 1) * 512)
                nc.tensor.matmul(pw[:, cs], lhsT=w1x_v,
                                 rhs=xc[:, 1 + 2 * g2:3 + 2 * g2, 1:1 + W],
                                 start=True, stop=False)
                nc.tensor.matmul(pw[:, cs], lhsT=w1k_v, rhs=kv[:, cs],
                                 start=False, stop=True)
                nc.scalar.activation(w1b[:, cs], pw[:, cs], AF.Relu)
            return w1b

        xc = xcp.tile([C, TR + 2, W + 2], BF16, tag="xc")
        nc.sync.dma_start(xc, xs.ap()[:, 0:TR + 2, :])
        w1b_cur = kemb_w1(0, xc)

        # ---------------- software-pipelined main loop ----------------
        for t in range(NT):
            if t < NT - 1:
                xc = xcp.tile([C, TR + 2, W + 2], BF16, tag="xc")
                nc.sync.dma_start(
                    xc, xs.ap()[:, TR * (t + 1):TR * (t + 1) + TR + 2, :])

            # wbar taps (tile_position-paired 64-contract MMs) + products
            prod = prodp.tile([C, 9216], BF16, tag="prod")

            def xsrc(tap):
                a, b = divmod(tap, 3)
                if b == 1:
                    return odd_cur[:, a:a + TR, :]
                return xv_slab[:, 4 * t + a:4 * t + a + TR, b:b + W]

            for jj in range(5):
                taps = [2 * jj] + ([2 * jj + 1] if jj < 4 else [])
                pbs = {}
                for ti, tap in enumerate(taps):
                    pb = pwb.tile([C, NPX], F32, tag="wbmm")
                    lo = 64 * ti
                    for h in range(2):
                        cs = slice(512 * h, 512 * h + 512)
                        nc.tensor.matmul(
                            pb[:, cs],
                            lhsT=we2_v[lo:lo + 64, jj, :],
                            rhs=w1b_cur[lo:lo + 64, cs],
                            start=True, stop=True,
                            tile_position=(lo, 0))
                    pbs[tap] = pb
                for tap in taps:
                    so = SLOT_OFF[tap]
                    pview = prod[:, so:so + NPX] \
                        .rearrange("p (r w) -> p r w", w=W)
                    pb3 = pbs[tap].rearrange("p (r w) -> p r w", w=W)
                    if tap in STT_TAPS:
                        nc.vector.scalar_tensor_tensor(
                            pview, pb3, be2_v[:, tap:tap + 1], xsrc(tap),
                            AL.add, AL.mult)
                    else:
                        wb = wbp.tile([C, NPX], BF16, tag="wb")
                        nc.scalar.activation(wb, pbs[tap], AF.Identity,
                                             bias=be2_v[:, tap:tap + 1])
                        nc.vector.tensor_tensor(
                            pview, wb.rearrange("p (r w) -> p r w", w=W),
                            xsrc(tap), AL.mult)
                if jj < 4:
                    # pair-add s_jj = A_jj + B_jj, interleaved with products
                    av = prod[:, jj * 1024:(jj + 1) * 1024]
                    nc.vector.tensor_tensor(av, av,
                                            prod[:, 4096 + jj * 1024:
                                                 4096 + (jj + 1) * 1024],
                                            AL.add)

            # mini-tree over the 4 contiguous pair sums + tap8
            nc.vector.tensor_tensor(prod[:, 0:2048], prod[:, 0:2048],
                                    prod[:, 2048:4096], AL.add)
            nc.vector.tensor_tensor(prod[:, 0:1024], prod[:, 0:1024],
                                    prod[:, 1024:2048], AL.add)
            nc.vector.tensor_tensor(prod[:, 0:1024], prod[:, 0:1024],
                                    prod[:, 8192:9216], AL.add)
            av = agg_slab[:, t * NPX:(t + 1) * NPX]
            nc.scalar.activation(av, prod[:, 0:1024], AF.Relu,
                                 accum_out=slots_a[:, t:t + 1])

            # tile t+1 kemb/w1 fills PE/ACT behind the tap phase
            if t < NT - 1:
                w1b_next = kemb_w1(t + 1, xc)

            # xv production for slab rows 4t+10..4t+13
            if t < 14:
                xin = xinp.tile([C, 4, W + 2], BF16, tag="xin")
                nc.sync.dma_start(xin, xs.ap()[:, 4 * t + 10:4 * t + 14, :])
                pv = pwb.tile([C, NPX], F32, tag="wbmm")
                nc.tensor.matmul(pv[:, 0:512], lhsT=wv_v,
                                 rhs=xin[:, 0:2, 1:1 + W],
                                 start=True, stop=True)
                nc.tensor.matmul(pv[:, 512:1024], lhsT=wv_v,
                                 rhs=xin[:, 2:4, 1:1 + W],
                                 start=True, stop=True)
                nc.scalar.activation(
                    xv_slab[:, 4 * t + 10:4 * t + 14, 1:1 + W],
                    pv.rearrange("p (r w) -> p r w", w=W), AF.Copy)

            # d = agg - kemb chunks, hidden in loop slack
            if t in (3, 7, 11):
                c0 = (t - 3) * 1024
                nc.vector.tensor_tensor(
                    agg_slab[:, c0:c0 + 4096], agg_slab[:, c0:c0 + 4096],
                    kemb_slab[:, c0:c0 + 4096], AL.subtract)
            elif t == 14:
                nc.vector.tensor_tensor(
                    agg_slab[:, 12288:15360], agg_slab[:, 12288:15360],
                    kemb_slab[:, 12288:15360], AL.subtract)
            elif t == 15:
                nc.vector.tensor_tensor(
                    agg_slab[:, 15360:16384], agg_slab[:, 15360:16384],
                    kemb_slab[:, 15360:16384], AL.subtract)

            if t < NT - 1:
                w1b_cur = w1b_next

            if t == 11:
                # early partial-gap AllReduce (tiles 0..11) -- hides peer skew
                sk1 = smallp.tile([C, 1], F32, tag="sk1")
                sa1 = smallp.tile([C, 1], F32, tag="sa1")
                nc.vector.tensor_reduce(sk1, slots_k[:, 0:24],
                                        mybir.AxisListType.X, AL.add)
                nc.vector.tensor_reduce(sa1, slots_a[:, 0:12],
                                        mybir.AxisListType.X, AL.add)
                gap1 = smallp.tile([C, 1], F32, tag="gap1")
                nc.vector.tensor_tensor(gap1, sk1, sa1, AL.add)
                nc.sync.dma_start(cc1_in.ap(), gap1)
                nc.gpsimd.collective_compute(
                    "AllReduce", AL.add,
                    replica_groups=[[0, 1, 2, 3], [4, 5, 6, 7]],
                    ins=[cc1_in.ap().opt()],
                    outs=[cc1_out.ap().opt()],
                )

            # odd-shifted xv copy for next tile
            if t < NT - 1:
                odd_cur = oddp.tile([C, 6, W], BF16, tag="odd")
                nc.sync.dma_start(
                    odd_cur, xv_slab[:, 4 * (t + 1):4 * (t + 1) + 6, 1:1 + W])

        # ---------------- gap remainder + final collective ----------------
        sum_k = smallp.tile([C, 1], F32, tag="sk")
        sum_a = smallp.tile([C, 1], F32, tag="sa")
        nc.vector.tensor_reduce(sum_k, slots_k[:, 24:32], mybir.AxisListType.X,
                                AL.add)
        nc.vector.tensor_reduce(sum_a, slots_a[:, 12:16], mybir.AxisListType.X,
                                AL.add)
        gap = smallp.tile([C, 1], F32, tag="gap")
        nc.vector.tensor_tensor(gap, sum_k, sum_a, AL.add)
        nc.sync.dma_start(cc_in.ap(), gap)
        nc.gpsimd.collective_compute(
            "AllReduce", AL.add,
            replica_groups=[[0, 1, 2, 3], [4, 5, 6, 7]],
            ins=[cc_in.ap().opt()],
            outs=[cc_out.ap().opt()],
        )
        gap2 = smallp.tile([C, 1], F32, tag="gap2")
        nc.sync.dma_start(gap2, cc_out.ap())
        gap1o = smallp.tile([C, 1], F32, tag="gap1o")
        nc.sync.dma_start(gap1o, cc1_out.ap())
        nc.vector.tensor_tensor(gap2, gap2, gap1o, AL.add)

        # ---------------- SE attention (tiny) ----------------
        ph = pkp.tile([64, 1], F32, tag="pk")
        nc.tensor.matmul(ph, lhsT=ws1_v, rhs=gap2, start=True, stop=True)
        hso = smallp.tile([64, 1], F32, tag="h")
        nc.scalar.activation(hso, ph, AF.Relu, bias=bs1_v)
        pa = pkp.tile([C, 2], F32, tag="pk")
        nc.tensor.matmul(pa[:, 0:1], lhsT=ws2_v[:, 0, :], rhs=hso,
                         start=True, stop=True)
        nc.tensor.matmul(pa[:, 1:2], lhsT=ws2_v[:, 1, :], rhs=hso,
                         start=True, stop=True)
        a01 = smallp.tile([C, 2], F32, tag="a01")
        nc.scalar.activation(a01[:, 0:1], pa[:, 0:1], AF.Identity,
                             bias=bs2_v[:, 0:1])
        nc.scalar.activation(a01[:, 1:2], pa[:, 1:2], AF.Identity,
                             bias=bs2_v[:, 1:2])
        dse = smallp.tile([C, 1], F32, tag="dse")
        nc.vector.tensor_tensor(dse, a01[:, 0:1], a01[:, 1:2], AL.subtract)
        nc.scalar.activation(attn0, dse, AF.Sigmoid)

        # ---------------- phase 2: out = kemb + attn0 * d ----------------
        # agg_slab holds d; scale by attn0 (4x ts, in place), add kemb,
        # store via SWDGE cast bf16->f32
        for c8 in range(8):
            cs = slice(c8 * 2048, (c8 + 1) * 2048)
            nc.vector.tensor_scalar(agg_slab[:, cs], agg_slab[:, cs],
                                    attn0[:, 0:1], None, AL.mult)
            ob = outp.tile([C, 2048], BF16, tag="ob")
            nc.vector.tensor_tensor(ob, agg_slab[:, cs], kemb_slab[:, cs],
                                    AL.add)
            nc.gpsimd.dma_start(out_d.ap()[:, cs], ob)

    return nc


_CACHE = {}


def _get_nc():
    if "nc" not in _CACHE:
        nc = bacc.Bacc("TRN2", target_bir_lowering=False, debug=False,
                       num_devices=NCORES)
        _build_kernel(nc)
        nc.compile()
        _CACHE["nc"] = nc
    return _CACHE["nc"]


def make_in_maps(inputs):
    x = np.asarray(inputs["x"], np.float32)
    wts = _prep_weights(inputs)
    xp = np.pad(x, ((0, 0), (0, 0), (1, 1), (1, 1))).astype(BF)
    in_maps = []
    for core in range(NCORES):
        bb, q = divmod(core, 4)
        slab = np.ascontiguousarray(xp[bb, :, RQ * q:RQ * q + RQ + 2, :])
        m = {"xs": slab}
        m.update(wts)
        in_maps.append(m)
    return in_maps


def kernel(**inputs):
    in_maps = make_in_maps(inputs)
    nc = _get_nc()
    res = run_bass_kernel_spmd(nc, in_maps, core_ids=list(range(NCORES)))
    out = np.empty((B, C, H, W), np.float32)
    for core in range(NCORES):
        bb, q = divmod(core, 4)
        out[bb, :, RQ * q:RQ * q + RQ, :] = \
            res.results[core]["out"].reshape(C, RQ, W)
    return out


# revision 9
# speedup vs baseline: 1.1528x; 1.1451x over previous
"""Trainium2 Bass kernel for nn_CotLayer (CoT attention layer), v6.

Computation (see reference):
  kemb = relu(grouped_conv3x3(x, Wk, groups=4))
  w1   = relu(We1 @ [x; kemb])            (1x1)
  wbar_k = We2_k @ w1 + be2_k             (per-pixel 3x3 kernel, SP-fold in We2_k)
  xv   = Wv @ x                           (1x1)
  agg  = relu(sum_k shift_k(xv) * wbar_k)
  gap  = mean_{H,W}(agg + kemb)           (AllReduce across 4-core groups)
  attn0 = sigmoid(SE-MLP delta)
  out  = kemb + attn0 * (agg - kemb)

Sharding: 8 cores = (batch b) x (H-quarter q), 64 output rows per core,
1-px halo baked into the input slab host-side; bf16 matmuls, fp32 PSUM.

Per macro-tile (4 rows x 256 cols = 1024 px), software-pipelined: tile t's
tap/product/tree phase runs while tile t+1's kemb/w1 matmuls+relus fill in
behind it (taps first in program order so the PE queue serves them early).
  PE : 18 kemb MM-512 (lhsT reused between row-halves), 4 w1 MM-512,
       9 wbar 64-contract MMs in tile_position-paired slots, 2 xv MM-512.
  ACT: kemb/w1 half-relus (+gap accum), xv PSUM->slab copy, agg relu
       (+gap accum), 8 wbar PSUM->SBUF bias copies.
  DVE: 9 products (1 via STT straight from PSUM, 8 via bf16 TT@2x),
       4 interleaved pair-adds + 3-op mini-tree, d=agg-kemb chunks
       (hidden in loop slack), phase-2 scale+add.
  DMA: x chunks, xv input chunks, odd-column shifted xv copies (SBUF->SBUF),
       SWDGE bf16->f32 cast stores.
Collectives: dummy AllReduce at startup warms the CC rings; the gap
AllReduce is split (tiles 0-11 issued early to hide peer skew + remainder).
"""

import numpy as np
import ml_dtypes
from contextlib import ExitStack

import concourse.bass as bass
import concourse.tile as tile
from concourse import bacc, mybir
from concourse.bass_utils import run_bass_kernel_spmd

F32 = mybir.dt.float32
BF16 = mybir.dt.bfloat16
AL = mybir.AluOpType
AF = mybir.ActivationFunctionType
BF = ml_dtypes.bfloat16

B, C, H, W = 2, 128, 256, 256
KSZ, SP = 3, 8
NCORES = 8
RQ = H // 4          # 64 rows per core
TR = 4               # output rows per macro-tile
NT = RQ // TR        # 16 macro-tiles per core
NPX = TR * W         # 1024 px per macro-tile

STT_TAPS = (1,)      # taps whose product reads wbar directly from PSUM
# product slot offsets in the [C, 9216] buffer: pair jj -> (A_jj, B_jj),
# tap8 -> 8192; pair-adds A_jj += B_jj land s_jj contiguously in [0:4096]
SLOT_OFF = {0: 0, 1: 4096, 2: 1024, 3: 5120, 4: 2048, 5: 6144,
            6: 3072, 7: 7168, 8: 8192}


def _prep_weights(inputs):
    Wk = np.asarray(inputs["Wk"], np.float32)
    We1 = np.asarray(inputs["We1"], np.float32)[:, :, 0, 0]
    We2 = np.asarray(inputs["We2"], np.float32)[:, :, 0, 0]
    be2 = np.asarray(inputs["be2"], np.float32)
    Wv = np.asarray(inputs["Wv"], np.float32)[:, :, 0, 0]
    Ws1 = np.asarray(inputs["Ws1"], np.float32)[:, :, 0, 0]
    bs1 = np.asarray(inputs["bs1"], np.float32)
    Ws2 = np.asarray(inputs["Ws2"], np.float32)[:, :, 0, 0]
    bs2 = np.asarray(inputs["bs2"], np.float32)

    # grouped conv as block-diag [C, 9, C]
    wk = np.zeros((C, 9, C), np.float32)
    for t in range(9):
        a, b = divmod(t, 3)
        for g in range(4):
            blk = Wk[32 * g:32 * g + 32, :, a, b]
            wk[32 * g:32 * g + 32, t, 32 * g:32 * g + 32] = blk.T
    # per-tap We2 with the SP-fold replication, taps paired into 64-row groups
    cidx = (np.arange(C) // SP) * 9
    we2 = np.zeros((64, 9, C), np.float32)
    be2k = np.zeros((C, 9), np.float32)
    for t in range(9):
        we2[:, t, :] = We2[cidx + t, :].T
        be2k[:, t] = be2[cidx + t]
    we2p = np.zeros((C, 5, C), np.float32)
    for jj in range(5):
        we2p[0:64, jj, :] = we2[:, 2 * jj, :]
        if jj < 4:
            we2p[64:C, jj, :] = we2[:, 2 * jj + 1, :]
    # w1 weights with duplicated output halves (rows 0-63 == rows 64-127)
    w1x2 = np.concatenate([We1[:, :C].T, We1[:, :C].T], axis=1)
    w1k2 = np.concatenate([We1[:, C:].T, We1[:, C:].T], axis=1)
    # SE weights
    ws1 = np.ascontiguousarray(Ws1.T / float(H * W))            # [C, 64]
    ws2 = np.zeros((64, 2, C), np.float32)
    ws2[:, 0, :] = Ws2[0::2, :].T
    ws2[:, 1, :] = Ws2[1::2, :].T
    bs2r = np.zeros((C, 2), np.float32)
    bs2r[:, 0] = bs2[0::2]
    bs2r[:, 1] = bs2[1::2]
    bs1c = np.zeros((C, 1), np.float32)
    bs1c[0:64, 0] = bs1

    wpack = np.concatenate([
        wk.reshape(C, 9 * C), w1x2, w1k2, we2p.reshape(C, 5 * C), Wv.T,
    ], axis=1).astype(BF)                                       # [C, 17C]
    fpack = np.zeros((C, 332), np.float32)
    fpack[:, 0:9] = be2k
    fpack[:, 9:73] = ws1
    fpack[:, 73:74] = bs1c
    fpack[:, 74:76] = bs2r
    fpack[0:64, 76:332] = ws2.reshape(64, 2 * C)
    return dict(wpack=np.ascontiguousarray(wpack),
                fpack=np.ascontiguousarray(fpack))


def _build_kernel(nc):
    xs = nc.dram_tensor("xs", [C, RQ + 2, W + 2], BF16, kind="ExternalInput")
    wpack_d = nc.dram_tensor("wpack", [C, 17 * C], BF16, kind="ExternalInput")
    fpack_d = nc.dram_tensor("fpack", [C, 332], F32, kind="ExternalInput")
    out_d = nc.dram_tensor("out", [C, RQ * W], F32, kind="ExternalOutput")

    cc_in = nc.dram_tensor("cc_in", [C, 1], F32, kind="Internal")
    cc_out = nc.dram_tensor("cc_out", [C, 1], F32, kind="Internal")
    cc1_in = nc.dram_tensor("cc1_in", [C, 1], F32, kind="Internal")
    cc1_out = nc.dram_tensor("cc1_out", [C, 1], F32, kind="Internal")
    ccw_in = nc.dram_tensor("ccw_in", [C, 1], F32, kind="Internal")
    ccw_out = nc.dram_tensor("ccw_out", [C, 1], F32, kind="Internal")

    with tile.TileContext(nc) as tc, ExitStack() as ctx:
        singles = ctx.enter_context(tc.tile_pool(name="singles", bufs=1))
        xcp = ctx.enter_context(tc.tile_pool(name="xcp", bufs=4))
        xinp = ctx.enter_context(tc.tile_pool(name="xinp", bufs=3))
        oddp = ctx.enter_context(tc.tile_pool(name="oddp", bufs=3))
        w1p = ctx.enter_context(tc.tile_pool(name="w1p", bufs=2))
        wbp = ctx.enter_context(tc.tile_pool(name="wbp", bufs=3))
        prodp = ctx.enter_context(tc.tile_pool(name="prodp", bufs=2))
        outp = ctx.enter_context(tc.tile_pool(name="outp", bufs=4))
        smallp = ctx.enter_context(tc.tile_pool(name="smallp", bufs=1))
        # PSUM: pkp (1 buf) chains kemb->w1->xv allocs; pwb (3 bufs)
        # rotates the 9 wbar taps so next-tile tap MMs start early enough
        # to keep the PE HAM-warm
        pkp = ctx.enter_context(tc.tile_pool(name="pkp", bufs=1, space="PSUM"))
        pwb = ctx.enter_context(tc.tile_pool(name="pwb", bufs=3, space="PSUM"))

        wpack_sb = singles.tile([C, 17 * C], BF16, tag="wpack")
        nc.sync.dma_start(wpack_sb, wpack_d.ap())
        fpack_sb = singles.tile([C, 332], F32, tag="fpack")
        nc.sync.dma_start(fpack_sb, fpack_d.ap())

        wk_v = wpack_sb[:, 0:9 * C].rearrange("p (t c) -> p t c", c=C)
        w1x_v = wpack_sb[:, 9 * C:10 * C]
        w1k_v = wpack_sb[:, 10 * C:11 * C]
        we2_v = wpack_sb[:, 11 * C:16 * C].rearrange("p (j c) -> p j c", c=C)
        wv_v = wpack_sb[:, 16 * C:17 * C]
        be2_v = fpack_sb[:, 0:9]
        ws1_v = fpack_sb[:, 9:73]
        bs1_v = fpack_sb[0:64, 73:74]
        bs2_v = fpack_sb[:, 74:76]
        ws2_v = fpack_sb[0:64, 76:332].rearrange("p (j c) -> p j c", c=C)

        kemb_slab = singles.tile([C, NT * NPX], BF16, tag="kemb")
        agg_slab = singles.tile([C, NT * NPX], BF16, tag="agg")
        xv_slab = singles.tile([C, RQ + 2, W + 2], BF16, tag="xvs")
        xin0 = singles.tile([C, 10, W + 2], BF16, tag="xin0")
        slots_k = singles.tile([C, 2 * NT], F32, tag="slotk")
        slots_a = singles.tile([C, NT], F32, tag="slota")
        dslots = singles.tile([C, 5], F32, tag="dslots")
        attn0 = singles.tile([C, 1], F32, tag="attn0")

        # zero the xv column borders (aggregation zero-pad)
        nc.gpsimd.memset(xv_slab[:, :, 0:1], 0.0)
        nc.gpsimd.memset(xv_slab[:, :, W + 1:W + 2], 0.0)

        # pre-warm sigmoid table + dummy AllReduce to warm the CC rings
        warm = smallp.tile([C, 1], F32, tag="warm")
        nc.vector.memset(warm, 0.0)
        nc.scalar.activation(warm, warm, AF.Sigmoid)
        nc.sync.dma_start(ccw_in.ap(), warm)
        nc.gpsimd.collective_compute(
            "AllReduce", AL.add,
            replica_groups=[[0, 1, 2, 3], [4, 5, 6, 7]],
            ins=[ccw_in.ap().opt()],
            outs=[ccw_out.ap().opt()],
        )

        # ---- xv bootstrap: slab rows 0..9 ----
        nc.sync.dma_start(xin0, xs.ap()[:, 0:10, :])
        for (r0, nr) in ((0, 4), (4, 4), (8, 2)):
            pv = pwb.tile([C, NPX], F32, tag="wbmm")
            nc.tensor.matmul(pv[:, 0:nr * W // 2], lhsT=wv_v,
                             rhs=xin0[:, r0:r0 + nr // 2, 1:1 + W],
                             start=True, stop=True)
            nc.tensor.matmul(pv[:, nr * W // 2:nr * W], lhsT=wv_v,
                             rhs=xin0[:, r0 + nr // 2:r0 + nr, 1:1 + W],
                             start=True, stop=True)
            nc.scalar.activation(
                xv_slab[:, r0:r0 + nr, 1:1 + W],
                pv[:, 0:nr * W].rearrange("p (r w) -> p r w", w=W), AF.Copy)

        odd_cur = oddp.tile([C, 6, W], BF16, tag="odd")
        nc.sync.dma_start(odd_cur, xv_slab[:, 0:6, 1:1 + W])

        # kemb/w1 pieces for tile t+1, interleaved between tile t's tap pairs
        def kemb_half(t, xc, g2):
            if g2 == 0:
                pk = pkp.tile([C, NPX], F32, tag="pk")
            else:
                pk = kemb_half.pk
            kemb_half.pk = pk
            kv = kemb_slab[:, t * NPX:(t + 1) * NPX]
            for tap in range(9):
                a, b = divmod(tap, 3)
                nc.tensor.matmul(
                    pk[:, g2 * 512:(g2 + 1) * 512],
                    lhsT=wk_v[:, tap, :],
                    rhs=xc[:, 2 * g2 + a:2 * g2 + a + 2, b:b + W],
                    start=(tap == 0), stop=(tap == 8),
                )
            nc.scalar.activation(kv[:, g2 * 512:(g2 + 1) * 512],
                                 pk[:, g2 * 512:(g2 + 1) * 512], AF.Relu,
                                 accum_out=slots_k[:, 2 * t + g2:
                                                   2 * t + g2 + 1])

        def w1_stage(t, xc):
            kv = kemb_slab[:, t * NPX:(t + 1) * NPX]
            pw = pkp.tile([C, NPX], F32, tag="pk")
            w1b = w1p.tile([C, NPX], BF16, tag="w1")
            for g2 in range(2):
                cs = slice(g2 * 512, (g2 + 1) * 512)
                nc.tensor.matmul(pw[:, cs], lhsT=w1x_v,
                                 rhs=xc[:, 1 + 2 * g2:3 + 2 * g2, 1:1 + W],
                                 start=True, stop=False)
                nc.tensor.matmul(pw[:, cs], lhsT=w1k_v, rhs=kv[:, cs],
                                 start=False, stop=True)
                nc.scalar.activation(w1b[:, cs], pw[:, cs], AF.Relu)
            return w1b

        xc = xcp.tile([C, TR + 2, W + 2], BF16, tag="xc")
        nc.sync.dma_start(xc, xs.ap()[:, 0:TR + 2, :])
        kemb_half(0, xc, 0)
        kemb_half(0, xc, 1)
        w1b_cur = w1_stage(0, xc)

        # ---------------- software-pipelined main loop ----------------
        for t in range(NT):
            if t < NT - 1:
                xc = xcp.tile([C, TR + 2, W + 2], BF16, tag="xc")
                nc.sync.dma_start(
                    xc, xs.ap()[:, TR * (t + 1):TR * (t + 1) + TR + 2, :])

            prod = prodp.tile([C, 9216], BF16, tag="prod")

            def xsrc(tap):
                a, b = divmod(tap, 3)
                if b == 1:
                    return odd_cur[:, a:a + TR, :]
                return xv_slab[:, 4 * t + a:4 * t + a + TR, b:b + W]

            def tap_pair(jj):
                taps = [2 * jj] + ([2 * jj + 1] if jj < 4 else [])
                pbs = {}
                for ti, tap in enumerate(taps):
                    pb = pwb.tile([C, NPX], F32, tag="wbmm")
                    lo = 64 * ti
                    for h in range(2):
                        cs = slice(512 * h, 512 * h + 512)
                        nc.tensor.matmul(
                            pb[:, cs],
                            lhsT=we2_v[lo:lo + 64, jj, :],
                            rhs=w1b_cur[lo:lo + 64, cs],
                            start=True, stop=True,
                            tile_position=(lo, 0))
                    pbs[tap] = pb
                for tap in taps:
                    so = SLOT_OFF[tap]
                    pview = prod[:, so:so + NPX] \
                        .rearrange("p (r w) -> p r w", w=W)
                    pb3 = pbs[tap].rearrange("p (r w) -> p r w", w=W)
                    if tap in STT_TAPS:
                        nc.vector.scalar_tensor_tensor(
                            pview, pb3, be2_v[:, tap:tap + 1], xsrc(tap),
                            AL.add, AL.mult)
                    else:
                        wb = wbp.tile([C, NPX], BF16, tag="wb")
                        nc.scalar.activation(wb, pbs[tap], AF.Identity,
                                             bias=be2_v[:, tap:tap + 1])
                        nc.vector.tensor_tensor(
                            pview, wb.rearrange("p (r w) -> p r w", w=W),
                            xsrc(tap), AL.mult)
                if jj < 4:
                    # pair-add s_jj = A_jj + B_jj, interleaved with products
                    av = prod[:, jj * 1024:(jj + 1) * 1024]
                    nc.vector.tensor_tensor(av, av,
                                            prod[:, 4096 + jj * 1024:
                                                 4096 + (jj + 1) * 1024],
                                            AL.add)

            # tap pairs of tile t interleaved with kemb/w1 work for tile t+1
            tap_pair(0)
            if t < NT - 1:
                kemb_half(t + 1, xc, 0)
            tap_pair(1)
            tap_pair(2)
            if t < NT - 1:
                kemb_half(t + 1, xc, 1)
            tap_pair(3)
            if t < NT - 1:
                w1b_next = w1_stage(t + 1, xc)
            tap_pair(4)

            # mini-tree over the 4 contiguous pair sums + tap8
            nc.vector.tensor_tensor(prod[:, 0:2048], prod[:, 0:2048],
                                    prod[:, 2048:4096], AL.add)
            nc.vector.tensor_tensor(prod[:, 0:1024], prod[:, 0:1024],
                                    prod[:, 1024:2048], AL.add)
            nc.vector.tensor_tensor(prod[:, 0:1024], prod[:, 0:1024],
                                    prod[:, 8192:9216], AL.add)
            av = agg_slab[:, t * NPX:(t + 1) * NPX]
            nc.scalar.activation(av, prod[:, 0:1024], AF.Relu,
                                 accum_out=slots_a[:, t:t + 1])

            # xv production for slab rows 4t+10..4t+13
            if t < 14:
                xin = xinp.tile([C, 4, W + 2], BF16, tag="xin")
                nc.sync.dma_start(xin, xs.ap()[:, 4 * t + 10:4 * t + 14, :])
                pv = pkp.tile([C, NPX], F32, tag="pk")
                nc.tensor.matmul(pv[:, 0:512], lhsT=wv_v,
                                 rhs=xin[:, 0:2, 1:1 + W],
                                 start=True, stop=True)
                nc.tensor.matmul(pv[:, 512:1024], lhsT=wv_v,
                                 rhs=xin[:, 2:4, 1:1 + W],
                                 start=True, stop=True)
                nc.scalar.activation(
                    xv_slab[:, 4 * t + 10:4 * t + 14, 1:1 + W],
                    pv.rearrange("p (r w) -> p r w", w=W), AF.Copy)

            # d = agg - kemb chunks, hidden in loop slack
            if t in (3, 7, 11):
                c0 = (t - 3) * 1024
                nc.vector.tensor_tensor(
                    agg_slab[:, c0:c0 + 4096], agg_slab[:, c0:c0 + 4096],
                    kemb_slab[:, c0:c0 + 4096], AL.subtract)
            elif t == 14:
                nc.vector.tensor_tensor(
                    agg_slab[:, 12288:15360], agg_slab[:, 12288:15360],
                    kemb_slab[:, 12288:15360], AL.subtract)
            elif t == 15:
                nc.vector.tensor_tensor(
                    agg_slab[:, 15360:16384], agg_slab[:, 15360:16384],
                    kemb_slab[:, 15360:16384], AL.subtract)

            if t < NT - 1:
                w1b_cur = w1b_next

            if t == 11:
                # early partial-gap AllReduce (tiles 0..11) -- hides peer skew
                sk1 = smallp.tile([C, 1], F32, tag="sk1")
                sa1 = smallp.tile([C, 1], F32, tag="sa1")
                nc.vector.tensor_reduce(sk1, slots_k[:, 0:24],
                                        mybir.AxisListType.X, AL.add)
                nc.vector.tensor_reduce(sa1, slots_a[:, 0:12],
                                        mybir.AxisListType.X, AL.add)
                gap1 = smallp.tile([C, 1], F32, tag="gap1")
                nc.vector.tensor_tensor(gap1, sk1, sa1, AL.add)
                nc.sync.dma_start(cc1_in.ap(), gap1)
                nc.gpsimd.collective_compute(
                    "AllReduce", AL.add,
                    replica_groups=[[0, 1, 2, 3], [4, 5, 6, 7]],
                    ins=[cc1_in.ap().opt()],
                    outs=[cc1_out.ap().opt()],
                )

            # odd-shifted xv copy for next tile
            if t < NT - 1:
                odd_cur = oddp.tile([C, 6, W], BF16, tag="odd")
                nc.sync.dma_start(
                    odd_cur, xv_slab[:, 4 * (t + 1):4 * (t + 1) + 6, 1:1 + W])

        # ---------------- gap remainder + final collective ----------------
        sum_k = smallp.tile([C, 1], F32, tag="sk")
        sum_a = smallp.tile([C, 1], F32, tag="sa")
        nc.vector.tensor_reduce(sum_k, slots_k[:, 24:32], mybir.AxisListType.X,
                                AL.add)
        nc.vector.tensor_reduce(sum_a, slots_a[:, 12:16], mybir.AxisListType.X,
                                AL.add)
        gap = smallp.tile([C, 1], F32, tag="gap")
        nc.vector.tensor_tensor(gap, sum_k, sum_a, AL.add)
        nc.sync.dma_start(cc_in.ap(), gap)
        nc.gpsimd.collective_compute(
            "AllReduce", AL.add,
            replica_groups=[[0, 1, 2, 3], [4, 5, 6, 7]],
            ins=[cc_in.ap().opt()],
            outs=[cc_out.ap().opt()],
        )
        gap2 = smallp.tile([C, 1], F32, tag="gap2")
        nc.sync.dma_start(gap2, cc_out.ap())
        gap1o = smallp.tile([C, 1], F32, tag="gap1o")
        nc.sync.dma_start(gap1o, cc1_out.ap())
        nc.vector.tensor_tensor(gap2, gap2, gap1o, AL.add)

        # ---------------- SE attention (tiny) ----------------
        ph = pkp.tile([64, 1], F32, tag="pk")
        nc.tensor.matmul(ph, lhsT=ws1_v, rhs=gap2, start=True, stop=True)
        hso = smallp.tile([64, 1], F32, tag="h")
        nc.scalar.activation(hso, ph, AF.Relu, bias=bs1_v)
        pa = pkp.tile([C, 2], F32, tag="pk")
        nc.tensor.matmul(pa[:, 0:1], lhsT=ws2_v[:, 0, :], rhs=hso,
                         start=True, stop=True)
        nc.tensor.matmul(pa[:, 1:2], lhsT=ws2_v[:, 1, :], rhs=hso,
                         start=True, stop=True)
        a01 = smallp.tile([C, 2], F32, tag="a01")
        nc.scalar.activation(a01[:, 0:1], pa[:, 0:1], AF.Identity,
                             bias=bs2_v[:, 0:1])
        nc.scalar.activation(a01[:, 1:2], pa[:, 1:2], AF.Identity,
                             bias=bs2_v[:, 1:2])
        dse = smallp.tile([C, 1], F32, tag="dse")
        nc.vector.tensor_tensor(dse, a01[:, 0:1], a01[:, 1:2], AL.subtract)
        nc.scalar.activation(attn0, dse, AF.Sigmoid)

        # ---------------- phase 2: out = kemb + attn0 * d ----------------
        # agg_slab holds d; scale by attn0 (4x ts, in place), add kemb,
        # store via SWDGE cast bf16->f32
        for c8 in range(8):
            cs = slice(c8 * 2048, (c8 + 1) * 2048)
            nc.vector.tensor_scalar(agg_slab[:, cs], agg_slab[:, cs],
                                    attn0[:, 0:1], None, AL.mult)
            ob = outp.tile([C, 2048], BF16, tag="ob")
            nc.vector.tensor_tensor(ob, agg_slab[:, cs], kemb_slab[:, cs],
                                    AL.add)
            nc.gpsimd.dma_start(out_d.ap()[:, cs], ob)

    return nc


_CACHE = {}


def _get_nc():
    if "nc" not in _CACHE:
        nc = bacc.Bacc("TRN2", target_bir_lowering=False, debug=False,
                       num_devices=NCORES)
        _build_kernel(nc)
        nc.compile()
        _CACHE["nc"] = nc
    return _CACHE["nc"]


def make_in_maps(inputs):
    x = np.asarray(inputs["x"], np.float32)
    wts = _prep_weights(inputs)
    xp = np.pad(x, ((0, 0), (0, 0), (1, 1), (1, 1))).astype(BF)
    in_maps = []
    for core in range(NCORES):
        bb, q = divmod(core, 4)
        slab = np.ascontiguousarray(xp[bb, :, RQ * q:RQ * q + RQ + 2, :])
        m = {"xs": slab}
        m.update(wts)
        in_maps.append(m)
    return in_maps


def kernel(**inputs):
    in_maps = make_in_maps(inputs)
    nc = _get_nc()
    res = run_bass_kernel_spmd(nc, in_maps, core_ids=list(range(NCORES)))
    out = np.empty((B, C, H, W), np.float32)
    for core in range(NCORES):
        bb, q = divmod(core, 4)
        out[bb, :, RQ * q:RQ * q + RQ, :] = \
            res.results[core]["out"].reshape(C, RQ, W)
    return out
